# revision 24
# baseline (speedup 1.0000x reference)
"""Bass/Tile TRN2 kernel for nn_MessageAggregation.

Computes: s = sum_n e2[n]; out = leaky_relu((e1+s) @ W1.T + (e1*s) @ W2.T)

Sharding: data-parallel over batch B=8192 across 8 NeuronCores (1024 rows
per core); W1/W2 replicated. Per-core layout: SBUF [128 partitions, 1024
free]; partition p holds batch rows 8p..8p+7 (4 KB contiguous per
partition per DMA descriptor). The kernel is DMA-bound (~32 MB of
all_embeddings2 per core at ~400 GB/s; stream floor ~85 us), so the shape
of the head/tail around the stream is what matters.

Fold-free structure: the n-reduction is split DVE 49 / PE 14 slices. PE
transpose-accumulates its slices chunk-wise straight into a TRANSPOSED
PSUM accumulator (two bank-sized [128,512] fp32 tiles; one accumulation
group per bank: first transpose start=True lazily zeroes the bank, the
rest accumulate, the tail closing transposes stop=True). DVE accumulates
the rest in SBUF (fp32; with GpSimd idle in-stream there is no SBUF
contention - both engines streaming 3-operand SBUF ops alongside the DMA
writes slows the whole machine ~20-30%, measured, so GpSimd does NOTHING
until the stream tail). The tail then needs NO fold: sT[c] arrives by
accumulating T(s_dve[c]) onto the PE accumulator with the closing
transposes.

Stream tail: slices 60-62 arrive as singleton 512 KB tiles and are added
REGION-SPLIT - DVE columns 0-639 (chunks 0-4), GpSimd columns 640-1023
(chunks 5-7, into the same SBUF accumulator, disjoint columns) - so each
column region is final right behind the last deliveries. Slice 63
arrives as 8 per-chunk 64 KB DMAs (DVE chunks 0-4, GpSimd 5-7).

Tail: closing transposes per chunk (PE, staggered behind the chunk
adds), x2t muls on DVE, st copies on scalar (fp32 PSUM -> f32r SBUF, the
rounding the f32r matmul verifier requires), then matmuls accumulate
onto the head e1@W1.T PSUM group (reopened with start=False): chunks 0-3
quartered (ready first, final store only 128 KB), chunks 4-7 as one
512-wide half. Lrelu on scalar; stores issued from the idle sync engine.
A dummy head lrelu keeps the activation table resident (a mid-tail
ACT_TABLE_LOAD costs 1.3 us).

Free position f = j*128 + p maps to batch row 8p + j; the host gather
un-permutes with a reshape/transpose (not timed).
"""

import sys

for _p in ("/opt/trn_rl_repo",):
    if _p not in sys.path:
        sys.path.insert(0, _p)

import numpy as np

import concourse.bacc as bacc
import concourse.mybir as mybir
import concourse.tile as tile
from concourse.masks import make_identity
from concourse.bass_utils import run_bass_kernel_spmd

B, N, D = 8192, 64, 128
M = 8  # cores
BL = B // M  # 1024 rows per core
R = BL // 128  # chunks per core (8)
F = BL  # free width of the [128, F] working layout
H = F // 2
F32 = mybir.dt.float32
F32R = mybir.dt.float32r
NEG_SLOPE = 0.01
LRELU = mybir.ActivationFunctionType.Lrelu

# Slices 0..55: D -> DVE slice-add, P -> PE transpose-accumulate.
# Slices 56..59 one DVE tile; 60-62 singleton region-split tiles; 63 chunked.
PLAN = [(4, "DDPD")] * 14 + [(4, "DDDD")]
assert sum(g for g, _ in PLAN) == 60
assert sum(r.count("P") for _, r in PLAN) == 14

SPLIT = 640  # DVE owns cols [0,640) of late slices; GpSimd [640,1024)
DVE_CHUNKS = [0, 1, 2, 3, 4]
GPS_CHUNKS = [5, 6, 7]


def build(load_bufs: int = 7):
    nc = bacc.Bacc(
        "TRN2",
        target_bir_lowering=False,
        debug=False,
        enable_asserts=False,
        num_devices=M,
    )
    e1 = nc.dram_tensor("embedding1", [BL, D], F32, kind="ExternalInput").ap()
    e2 = nc.dram_tensor("all_embeddings2", [N, BL, D], F32, kind="ExternalInput").ap()
    w1 = nc.dram_tensor("W1", [D, D], F32, kind="ExternalInput").ap()
    w2 = nc.dram_tensor("W2", [D, D], F32, kind="ExternalInput").ap()
    out = nc.dram_tensor("out", [D, BL], F32, kind="ExternalOutput").ap()

    e1_r = e1.rearrange("(p r) d -> p (r d)", p=128)  # [128, 1024]
    e2_r = e2.rearrange("n (p r) d -> p n (r d)", p=128)  # [128, 64, 1024]

    with tile.TileContext(nc) as tc:
        with (
            tc.tile_pool(name="const", bufs=1) as cpool,
            tc.tile_pool(name="load", bufs=load_bufs) as lpool,
            tc.tile_pool(name="late", bufs=3) as latepool,
            tc.tile_pool(name="last", bufs=8) as lastpool,
            tc.tile_pool(name="act", bufs=1) as apool,
            tc.tile_pool(name="spet", bufs=1, space="PSUM") as spool,
            tc.tile_pool(name="ops", bufs=1, space="PSUM") as opool,
            tc.tile_pool(name="trps", bufs=2, space="PSUM") as trpool,
        ):
            ident = cpool.tile([128, 128], F32)
            make_identity(nc, ident[:])

            w1_sb = cpool.tile([128, 128], F32)
            nc.scalar.dma_start(out=w1_sb[:], in_=w1)
            w2_sb = cpool.tile([128, 128], F32)
            nc.scalar.dma_start(out=w2_sb[:], in_=w2)
            e1_sb = apool.tile([128, F], F32)
            nc.scalar.dma_start(out=e1_sb[:], in_=e1_r)

            # W.T in SBUF: stationary operand of the output matmuls. fp32
            # for the exact e1-term at the head; f32r for the single-pass
            # tail matmuls.
            w1t_ps = trpool.tile([128, 128], F32, tag="tr")
            nc.tensor.transpose(w1t_ps[:], w1_sb[:], ident[:])
            w1t = cpool.tile([128, 128], F32)
            nc.scalar.copy(out=w1t[:], in_=w1t_ps[:])
            w1t_r = cpool.tile([128, 128], F32R)
            nc.scalar.copy(out=w1t_r[:], in_=w1t_ps[:])
            w2t_ps = trpool.tile([128, 128], F32, tag="tr")
            nc.tensor.transpose(w2t_ps[:], w2_sb[:], ident[:])
            w2t_r = cpool.tile([128, 128], F32R)
            nc.scalar.copy(out=w2t_r[:], in_=w2t_ps[:])

            # Dummy lrelu at the head so its table is resident for the tail.
            warm = cpool.tile([128, 8], F32)
            nc.scalar.activation(warm[:], ident[:, 0:8], LRELU, alpha=NEG_SLOPE)

            # e1^T pre-stage: chunk j of e1 transposed -> e1t[:, j*128:(j+1)*128]
            e1t = apool.tile([128, F], F32)
            for j in range(R):
                sl = slice(j * 128, (j + 1) * 128)
                tp = trpool.tile([128, 128], F32, tag="tr")
                nc.tensor.transpose(tp[:], e1_sb[:, sl], ident[:])
                nc.scalar.copy(out=e1t[:, sl], in_=tp[:])

            # e1 @ W1.T term of out_T, as a CLOSED accumulation group per
            # half (the tail reopens with start=False).
            o_ps0 = opool.tile([128, H], F32)
            o_ps1 = opool.tile([128, H], F32)
            o_ps = [o_ps0, o_ps1]
            for h in range(2):
                hs = slice(h * H, (h + 1) * H)
                nc.tensor.matmul(
                    o_ps[h][:], lhsT=w1t[:], rhs=e1t[:, hs], start=True, stop=True
                )

            # Transposed PSUM accumulators: bank A = chunks 0-3, B = 4-7.
            spe_a = spool.tile([128, 512], F32, tag="speA")
            spe_b = spool.tile([128, 512], F32, tag="speB")
            spe = [spe_a, spe_b]
            started = [False, False]

            def tr_acc(src_chunk_ap, c, stop):
                bank = c // 4
                sub = slice((c % 4) * 128, (c % 4 + 1) * 128)
                nc.tensor.matmul(
                    spe[bank][:, sub],
                    lhsT=src_chunk_ap,
                    rhs=ident[:],
                    is_transpose=True,
                    start=not started[bank],
                    stop=stop,
                    skip_group_check=True,
                )
                started[bank] = True

            def tp_of(c):
                bank = c // 4
                sub = slice((c % 4) * 128, (c % 4 + 1) * 128)
                return spe[bank][:, sub]

            # ---- stream (slices 0..59) ----
            s_dve = apool.tile([128, F], F32)
            seen_d = 0
            base = 0
            for gl, routing in PLAN:
                t = lpool.tile([128, gl * F], F32, tag="load")
                nc.sync.dma_start(
                    out=t[:].rearrange("p (n f) -> p n f", n=gl),
                    in_=e2_r[:, base : base + gl, :],
                )
                for g in range(gl):
                    sl = t[:, g * F : (g + 1) * F]
                    if routing[g] == "D":
                        seen_d += 1
                        if seen_d == 1:
                            nc.vector.tensor_copy(out=s_dve[:], in_=sl)
                        else:
                            nc.vector.tensor_add(out=s_dve[:], in0=s_dve[:], in1=sl)
                    else:
                        for c in range(R):
                            tr_acc(
                                t[:, g * F + c * 128 : g * F + (c + 1) * 128],
                                c,
                                False,
                            )
                base += gl

            # Slices 60-62: singleton tiles, region-split adds.
            for n in (60, 61, 62):
                lt = latepool.tile([128, F], F32, tag=f"late{n}")
                nc.sync.dma_start(out=lt[:], in_=e2_r[:, n, :])
                nc.vector.tensor_add(
                    out=s_dve[:, 0:SPLIT], in0=s_dve[:, 0:SPLIT], in1=lt[:, 0:SPLIT]
                )
                nc.gpsimd.tensor_add(
                    out=s_dve[:, SPLIT:F], in0=s_dve[:, SPLIT:F], in1=lt[:, SPLIT:F]
                )

            # Slice 63: 8 per-chunk DMAs, staggered final adds.
            last_t = {}
            for c in DVE_CHUNKS + GPS_CHUNKS:
                tcch = lastpool.tile([128, 128], F32, tag=f"lc{c}")
                nc.sync.dma_start(
                    out=tcch[:], in_=e2_r[:, N - 1, c * 128 : (c + 1) * 128]
                )
                last_t[c] = tcch
            for c in DVE_CHUNKS:
                sl = slice(c * 128, (c + 1) * 128)
                nc.vector.tensor_add(
                    out=s_dve[:, sl], in0=s_dve[:, sl], in1=last_t[c][:]
                )
            for c in GPS_CHUNKS:
                sl = slice(c * 128, (c + 1) * 128)
                nc.gpsimd.tensor_add(
                    out=s_dve[:, sl], in0=s_dve[:, sl], in1=last_t[c][:]
                )

            # ---- tail: closing transposes (accumulate T(s_dve[c]) onto the
            # PE accumulator; the result IS sT[c]). No folds.
            st = apool.tile([128, F], F32R)
            x2t = apool.tile([128, F], F32R)
            out_sb = apool.tile([128, F], F32)

            for c in [0, 1, 2, 3, 4, 5, 6, 7]:
                sl = slice(c * 128, (c + 1) * 128)
                tr_acc(s_dve[:, sl], c, stop=(c % 4 == 3))

            for c in [0, 1, 2, 3, 4, 5, 6, 7]:
                sl = slice(c * 128, (c + 1) * 128)
                nc.vector.tensor_mul(out=x2t[:, sl], in0=e1t[:, sl], in1=tp_of(c))
            for c in [0, 1, 2, 3, 4, 5, 6, 7]:
                sl = slice(c * 128, (c + 1) * 128)
                nc.scalar.copy(out=st[:, sl], in_=tp_of(c))

            # h0 (chunks 0-3, ready first) quartered; h1 as one half.
            Q = H // 2
            for q in range(2):
                qs = slice(q * Q, (q + 1) * Q)
                ops_q = o_ps[0][:, qs]
                nc.tensor.matmul(
                    ops_q, lhsT=w1t_r[:], rhs=st[:, qs], start=False, stop=False
                )
                nc.tensor.matmul(
                    ops_q,
                    lhsT=w2t_r[:],
                    rhs=x2t[:, qs],
                    start=False,
                    stop=(q == 1),
                    skip_group_check=True,
                )
                nc.scalar.activation(out_sb[:, qs], ops_q, LRELU, alpha=NEG_SLOPE)
                nc.sync.dma_start(out=out[:, qs], in_=out_sb[:, qs])
            hs = slice(H, F)
            nc.tensor.matmul(
                o_ps[1][:], lhsT=w1t_r[:], rhs=st[:, hs], start=False, stop=False
            )
            nc.tensor.matmul(
                o_ps[1][:], lhsT=w2t_r[:], rhs=x2t[:, hs], start=False, stop=True
            )
            nc.scalar.activation(out_sb[:, hs], o_ps[1][:], LRELU, alpha=NEG_SLOPE)
            nc.sync.dma_start(out=out[:, hs], in_=out_sb[:, hs])

    nc.compile()
    return nc


_NC = None


def _get_nc():
    global _NC
    if _NC is None:
        _NC = build()
    return _NC


def _make_in_maps(inputs):
    e1 = np.asarray(inputs["embedding1"], dtype=np.float32)
    e2 = np.asarray(inputs["all_embeddings2"], dtype=np.float32)
    w1 = np.asarray(inputs["W1"], dtype=np.float32)
    w2 = np.asarray(inputs["W2"], dtype=np.float32)
    in_maps = []
    for k in range(M):
        sl = slice(k * BL, (k + 1) * BL)
        in_maps.append(
            {
                "embedding1": np.ascontiguousarray(e1[sl]),
                "all_embeddings2": np.ascontiguousarray(e2[:, sl, :]),
                "W1": w1,
                "W2": w2,
            }
        )
    return in_maps


def _unshard(arr):
    # arr: out_T [o=128, f=1024] with f = j*128 + p <-> batch row 8p + j
    return arr.reshape(128, 8, 128).transpose(2, 1, 0).reshape(BL, D)


def _run(inputs, trace=False, **kwargs):
    nc = _get_nc()
    res = run_bass_kernel_spmd(
        nc, _make_in_maps(inputs), core_ids=list(range(M)), trace=trace, **kwargs
    )
    full = np.concatenate(
        [_unshard(res.results[k]["out"]) for k in range(M)], axis=0
    )
    return full, res


def kernel(**inputs):
    full, _ = _run(inputs)
    return full
